# revision 14
# baseline (speedup 1.0000x reference)
"""Trainium2 Bass kernel for the Luong-attention LSTM decoder (nn_Decoder).

8-core strategy:
- Host folds Wa into the recurrence: attn@Wk_a = h2@R' + ctx@C' with
  R' = Wa_top@Wk_a + lstm_r, C' = Wa_bot@Wk_a. The x-projection (+bias) is
  host-precomputed. attn2/logits defer to a batched epilogue.
- Recurrence replicated on all cores (weight-stream-bound); state kept
  transposed (h2T/ctxT) as matmul lhsT; gates pipelined in h-quarters.
- Attention batch-sharded (8 batches/core); one AllGather per step
  reassembles ctxT.
- Epilogue: attn2 = [h2,ctx]@Wa, logits = attn2@fc_w with vocab column-sharded
  4000/core; host concatenates vocab shards.
"""
import sys

sys.path.insert(0, "/opt/trn_rl_repo")

import numpy as np
import concourse.bass as bass
import concourse.tile as tile
from concourse import bacc, mybir
from concourse.bass_utils import run_bass_kernel_spmd

B, T_IN, T_DEC = 64, 64, 47
V, E, H = 32000, 512, 1024
NC = 8
BL = B // NC
VS = V // NC
KC = H // 128
NPAIR = BL // 2
GS = VS // 8
F32 = mybir.dt.float32
BF16 = mybir.dt.bfloat16

_CACHE = {}


def _build():
    nc = bacc.Bacc(None, target_bir_lowering=False)

    wz_d = nc.dram_tensor("wz", [16, 128, 4096], BF16, kind="ExternalInput")
    xp_d = nc.dram_tensor("xp", [T_DEC, B, 4096], BF16, kind="ExternalInput")
    h2t0_d = nc.dram_tensor("h2t0", [KC, 128, 72], BF16, kind="ExternalInput")
    c0_d = nc.dram_tensor("c0", [B, H], F32, kind="ExternalInput")
    wm_d = nc.dram_tensor("wm", [KC, 128, H], BF16, kind="ExternalInput")
    memo_d = nc.dram_tensor("memo", [BL, T_IN, H], BF16, kind="ExternalInput")
    isel_d = nc.dram_tensor("isel", [B, 72], F32, kind="ExternalInput")
    wa_d = nc.dram_tensor("wa", [16, 128, H], BF16, kind="ExternalInput")
    fcw_d = nc.dram_tensor("fcw", [KC, 128, VS], BF16, kind="ExternalInput")
    fcbr_d = nc.dram_tensor("fcbr", [128, VS], F32, kind="ExternalInput")
    idb_d = nc.dram_tensor("idb", [128, 128], BF16, kind="ExternalInput")
    out_d = nc.dram_tensor("out", [B, T_DEC, VS], F32, kind="ExternalOutput")

    h2h_d = nc.dram_tensor("h2hist", [T_DEC, 128, KC * B], BF16)
    ctxh_d = nc.dram_tensor("ctxhist", [T_DEC, 128, NC * KC * BL], BF16)
    agin = [nc.dram_tensor(f"agin{t}", [128, KC * BL], BF16) for t in range(T_DEC)]
    agout = [
        nc.dram_tensor(f"agout{t}", [NC * 128, KC * BL], BF16, addr_space="Shared")
        for t in range(T_DEC)
    ]
    rg = [list(range(NC))]

    with tile.TileContext(nc) as tc:
        with (
            tc.tile_pool(name="one", bufs=1) as one,
            tc.tile_pool(name="work", bufs=2) as work,
            tc.tile_pool(name="gat", bufs=2) as gat,
            tc.tile_pool(name="zps", bufs=2, space="PSUM") as zps,
            tc.tile_pool(name="tps", bufs=2, space="PSUM") as tps,
            tc.tile_pool(name="aps", bufs=2, space="PSUM") as aps,
        ):
            # ---------------- resident tiles ----------------
            # h2tx: [128, chunk, 72]: cols 0:64 full h2T, 64:72 own-batch h2T
            h2tx = one.tile([128, KC, 72], BF16, tag="h2tx")
            nc.gpsimd.dma_start(h2tx[:], h2t0_d.rearrange("c p b -> p c b"))
            isel = one.tile([B, 72], F32, tag="isel")
            nc.gpsimd.dma_start(isel[:], isel_d[:])
            ctxt = one.tile([128, KC, NC, BL], BF16, tag="ctxt")
            nc.vector.memset(ctxt[:], 0.0)
            cst = one.tile([B, H], F32, tag="cst")
            nc.gpsimd.dma_start(cst[:], c0_d[:])

            # keys for own batches: keysK[:, c2, b, t]
            keysK = one.tile([128, KC, BL, T_IN], BF16)
            with (
                tc.tile_pool(name="boot", bufs=1) as boot,
                tc.tile_pool(name="bootw", bufs=2) as bootw,
            ):
                memT = boot.tile([128, KC, BL * T_IN], BF16)
                for c in range(KC):
                    for b in range(BL):
                        nc.sync.dma_start_transpose(
                            memT[:, c, b * T_IN:(b + 1) * T_IN],
                            memo_d[b, :, c * 128:(c + 1) * 128],
                        )
                for c2 in range(KC):
                    kp = aps.tile([128, BL * T_IN], F32, tag="a")
                    for k in range(KC):
                        wmc = bootw.tile([128, 128], BF16, tag="wmc")
                        nc.gpsimd.dma_start(wmc[:], wm_d[k, :, c2 * 128:(c2 + 1) * 128])
                        nc.tensor.matmul(
                            kp[:], wmc[:], memT[:, k],
                            start=(k == 0), stop=(k == KC - 1),
                        )
                    nc.scalar.copy(keysK[:, c2], kp[:].rearrange("p (b t) -> p b t", b=BL))

            wz = one.tile([128, 16, 4096], BF16)
            for k in range(16):
                nc.gpsimd.dma_start(wz[:, k], wz_d[k])

            # mem pair-packed for ctx: memPK[(parity*64+t), pair, chunk, h]
            memPK = one.tile([128, NPAIR, KC, 128], BF16)
            for b in range(BL):
                half = (b % 2) * 64
                nc.gpsimd.dma_start(
                    memPK[half:half + 64, b // 2].rearrange("t c h -> t (c h)"),
                    memo_d[b],
                )

            ones64 = one.tile([B, 1], F32, tag="ones64")
            nc.vector.memset(ones64[:], 1.0)
            onesr = one.tile([1, B], F32, tag="onesr")
            nc.vector.memset(onesr[:], 1.0)
            alignZ = one.tile([128, BL], BF16, tag="alignZ")
            nc.vector.memset(alignZ[:], 0.0)

            # ================= decode loop =================
            for t in range(T_DEC):
                xpt = work.tile([B, 4096], BF16, tag="xp")
                nc.gpsimd.dma_start(xpt[:], xp_d[t])
                h2b = work.tile([B, H], F32, tag="h2b")
                for q in range(4):  # h-quarters of 256
                    zq = zps.tile([B, 1024], F32, tag="zq")
                    for k in range(16):
                        if k < KC:
                            lhs = h2tx[:, k, 0:64]
                        else:
                            lhs = ctxt[:, k - KC].rearrange("p r w -> p (r w)")
                        nc.tensor.matmul(
                            zq[:, 0:512], lhs,
                            wz[:, k, q * 1024:q * 1024 + 512],
                            start=(k == 0), stop=(k == 15),
                        )
                        nc.tensor.matmul(
                            zq[:, 512:1024], lhs,
                            wz[:, k, q * 1024 + 512:(q + 1) * 1024],
                            start=(k == 0), stop=(k == 15),
                        )
                    # z2 = z + xproj ; gate order within zq: i,f,g,o
                    z2 = gat.tile([B, 4, 256], F32, tag="z2")
                    nc.vector.scalar_tensor_tensor(
                        z2[:], zq[:].rearrange("b (g n) -> b g n", g=4),
                        1.0, xpt[:, q * 1024:(q + 1) * 1024].rearrange("b (g n) -> b g n", g=4),
                        mybir.AluOpType.mult, mybir.AluOpType.add,
                    )
                    sif = gat.tile([B, 512], F32, tag="sif")
                    nc.scalar.activation(
                        sif[:].rearrange("b (a n) -> b a n", a=2), z2[:, 0:2],
                        mybir.ActivationFunctionType.Sigmoid)
                    so = gat.tile([B, 256], F32, tag="so")
                    nc.scalar.activation(so[:], z2[:, 3],
                                         mybir.ActivationFunctionType.Sigmoid)
                    tg = gat.tile([B, 256], F32, tag="tg")
                    nc.scalar.activation(tg[:], z2[:, 2],
                                         mybir.ActivationFunctionType.Tanh)
                    qs = slice(q * 256, (q + 1) * 256)
                    t1 = gat.tile([B, 256], F32, tag="t1")
                    nc.vector.tensor_mul(t1[:], sif[:, 256:512], cst[:, qs])
                    t2 = gat.tile([B, 256], F32, tag="t2")
                    nc.vector.tensor_mul(t2[:], sif[:, 0:256], tg[:])
                    nc.vector.tensor_add(cst[:, qs], t1[:], t2[:])
                    th = gat.tile([B, 256], F32, tag="th")
                    nc.scalar.activation(th[:], cst[:, qs],
                                         mybir.ActivationFunctionType.Tanh)
                    nc.vector.tensor_mul(h2b[:, qs], so[:], th[:])

                # transpose h2 (+ own-col gather): [64,128] @ [64,72]
                for c in range(KC):
                    tp = tps.tile([128, 72], F32, tag="tp")
                    nc.tensor.matmul(tp[:], h2b[:, c * 128:(c + 1) * 128], isel[:],
                                     start=True, stop=True)
                    nc.scalar.copy(h2tx[:, c, :], tp[:])
                nc.gpsimd.dma_start(
                    h2h_d[t].rearrange("p (c b) -> p c b", c=KC), h2tx[:, :, 0:64]
                )

                # ---- score (own batches): scT8[t, j] ----
                scT8 = aps.tile([64, BL], F32, tag="a")
                for j in range(BL):
                    for c in range(KC):
                        nc.tensor.matmul(
                            scT8[:, j:j + 1], keysK[:, c, j, :],
                            h2tx[:, c, 64 + j:64 + j + 1],
                            start=(c == 0), stop=(c == KC - 1),
                        )
                e8 = gat.tile([64, BL], F32, tag="e8")
                nc.scalar.activation(e8[:], scT8[:], mybir.ActivationFunctionType.Exp)
                s18 = tps.tile([1, BL], F32, tag="tp")
                nc.tensor.matmul(s18[:], ones64[:], e8[:], start=True, stop=True)
                r18 = gat.tile([1, BL], F32, tag="r18")
                nc.vector.reciprocal(r18[:], s18[:])
                rb = tps.tile([64, BL], F32, tag="tp")
                nc.tensor.matmul(rb[:], onesr[:], r18[:], start=True, stop=True)
                a8 = gat.tile([64, BL], BF16, tag="a8")
                nc.vector.tensor_mul(a8[:], e8[:], rb[:])
                # scatter: even own-batches -> upper half, odd -> lower half
                nc.vector.tensor_copy(
                    alignZ[0:64, :].rearrange("p (pr two) -> p pr two", two=2)[:, :, 0],
                    a8[:].rearrange("p (pr two) -> p pr two", two=2)[:, :, 0],
                )
                nc.vector.tensor_copy(
                    alignZ[64:128, :].rearrange("p (pr two) -> p pr two", two=2)[:, :, 1],
                    a8[:].rearrange("p (pr two) -> p pr two", two=2)[:, :, 1],
                )

                # ---- ctx (own batches, pair-packed block-diag) ----
                ctxPS = aps.tile([128, KC, BL], F32, tag="a")
                for pr in range(NPAIR):
                    for c in range(KC):
                        nc.tensor.matmul(
                            ctxPS[:, c, 2 * pr:2 * pr + 2],
                            memPK[:, pr, c, :],
                            alignZ[:, 2 * pr:2 * pr + 2],
                            start=True, stop=True,
                        )
                ctxo = gat.tile([128, KC, BL], BF16, tag="ctxo")
                nc.scalar.copy(ctxo[:], ctxPS[:])

                # ---- AllGather ctx ----
                nc.gpsimd.dma_start(agin[t][:], ctxo[:].rearrange("p c w -> p (c w)"))
                nc.gpsimd.collective_compute(
                    "AllGather", mybir.AluOpType.bypass,
                    replica_groups=rg,
                    ins=[agin[t][:]], outs=[agout[t][:]],
                )
                for r in range(NC):
                    nc.gpsimd.dma_start(
                        ctxt[:, :, r, :],
                        agout[t][r * 128:(r + 1) * 128, :].rearrange("p (c w) -> p c w", c=KC),
                    )
                nc.gpsimd.dma_start(
                    ctxh_d[t], ctxt[:].rearrange("p c r w -> p (c r w)")
                )

        # ================= epilogue =================
        with (
            tc.tile_pool(name="eone", bufs=1) as eone,
            tc.tile_pool(name="ework", bufs=3) as ework,
            tc.tile_pool(name="eps", bufs=2, space="PSUM") as eps,
            tc.tile_pool(name="fps", bufs=2, space="PSUM") as fps,
        ):
            wa_sb = eone.tile([128, 16, H], BF16)
            nc.gpsimd.dma_start(wa_sb[:], wa_d.rearrange("k p h -> p k h"))
            fcw_sb = eone.tile([128, KC, VS], BF16)
            nc.gpsimd.dma_start(fcw_sb[:], fcw_d.rearrange("k p v -> p k v"))
            identb = eone.tile([128, 128], BF16)
            nc.gpsimd.dma_start(identb[:], idb_d[:])
            fcbR = eone.tile([128, VS], F32)
            nc.gpsimd.dma_start(fcbR[:], fcbr_d[:])

            for p in range((T_DEC + 1) // 2):
                t0 = 2 * p
                nsteps = 2 if t0 + 1 < T_DEC else 1
                M = 64 * nsteps
                h2p = ework.tile([128, KC, 2, B], BF16, tag="h2p")
                ctxp = ework.tile([128, KC, 2, B], BF16, tag="ctxp")
                for i in range(nsteps):
                    nc.gpsimd.dma_start(
                        h2p[:, :, i, :],
                        h2h_d[t0 + i].rearrange("p (c b) -> p c b", c=KC),
                    )
                    nc.gpsimd.dma_start(
                        ctxp[:, :, i, :],
                        ctxh_d[t0 + i].rearrange("p (c b) -> p c b", c=KC),
                    )
                a2 = eps.tile([128, H], F32, tag="a2")
                for k in range(16):
                    if k < KC:
                        lhs = h2p[:, k, 0:nsteps, :].rearrange("p s b -> p (s b)")
                    else:
                        lhs = ctxp[:, k - KC, 0:nsteps, :].rearrange("p s b -> p (s b)")
                    for n in range(2):
                        nc.tensor.matmul(
                            a2[0:M, n * 512:(n + 1) * 512],
                            lhs, wa_sb[:, k, n * 512:(n + 1) * 512],
                            start=(k == 0), stop=(k == 15),
                        )
                a2sb = ework.tile([128, H], BF16, tag="a2sb")
                nc.scalar.copy(a2sb[0:M, :], a2[0:M, :])
                a2t = ework.tile([128, KC, 128], BF16, tag="a2t")
                for c2 in range(KC):
                    tp = eps.tile([128, 128], BF16, tag="a2tp")
                    nc.tensor.transpose(
                        tp[:, 0:M], a2sb[0:M, c2 * 128:(c2 + 1) * 128], identb[0:M, 0:M]
                    )
                    nc.scalar.copy(a2t[:, c2, 0:M], tp[:, 0:M])
                for g in range(8):
                    lg = fps.tile([128, GS], F32, tag="lg")
                    for k in range(KC):
                        nc.tensor.matmul(
                            lg[0:M, :], a2t[:, k, 0:M],
                            fcw_sb[:, k, g * GS:(g + 1) * GS],
                            start=(k == 0), stop=(k == KC - 1),
                        )
                    lgs = ework.tile([128, GS], F32, tag="lgs")
                    nc.vector.scalar_tensor_tensor(
                        lgs[0:M, :], lg[0:M, :], 1.0, fcbR[0:M, g * GS:(g + 1) * GS],
                        mybir.AluOpType.mult, mybir.AluOpType.add,
                    )
                    for i in range(nsteps):
                        nc.gpsimd.dma_start(
                            out_d[:, t0 + i, g * GS:(g + 1) * GS],
                            lgs[i * 64:(i + 1) * 64, :],
                        )

    nc.finalize()
    return nc


def _prep_inputs(inputs):
    bfnp = mybir.dt.np(BF16)
    f32 = lambda x: np.asarray(x, dtype=np.float32)
    tokens = np.asarray(inputs["tokens"])
    memory = f32(inputs["memory"])
    enc_h = f32(inputs["enc_h"])
    enc_c = f32(inputs["enc_c"])
    emb = f32(inputs["emb"])
    Wm = f32(inputs["Wm"])
    Wa = f32(inputs["Wa"])
    lstm_k = f32(inputs["lstm_k"])
    lstm_r = f32(inputs["lstm_r"])
    lstm_b = f32(inputs["lstm_b"])
    fc_w = f32(inputs["fc_w"])
    fc_b = f32(inputs["fc_b"])

    Wk_x = lstm_k[:E]
    Wk_a = lstm_k[E:]
    Rp = Wa[:H] @ Wk_a + lstm_r
    Cp = Wa[H:] @ Wk_a
    wzf = np.concatenate([Rp, Cp], 0)
    wzf = wzf.reshape(2048, 4, 4, 256).transpose(0, 2, 1, 3).reshape(2048, 4096)
    wz = np.ascontiguousarray(wzf).reshape(16, 128, 4096).astype(bfnp)
    xs = emb[tokens]                                   # [B, T_DEC, E]
    xpb = xs @ Wk_x + lstm_b
    # t=0 folding correction: attn_0 = 0 (not [enc_h,0]@Wa) and h_0 = enc_h;
    # absorb enc_h@lstm_r into xproj[0] and start the device h2 state at zero.
    xpb[:, 0] += enc_h @ lstm_r
    xpf = xpb.transpose(1, 0, 2)
    xpf = xpf.reshape(T_DEC, B, 4, 4, 256).transpose(0, 1, 3, 2, 4)
    xp = np.ascontiguousarray(xpf).reshape(T_DEC, B, 4096).astype(bfnp)
    wm = Wm.reshape(KC, 128, H).astype(bfnp)
    wa = Wa.reshape(16, 128, H).astype(bfnp)
    idb = np.eye(128, dtype=np.float32).astype(bfnp)

    common = dict(wz=wz, xp=xp, wm=wm, wa=wa, c0=enc_c.copy(), idb=idb)
    h2t_full = np.zeros((KC, 128, B), np.float32)
    maps = []
    for r in range(NC):
        own = slice(r * BL, (r + 1) * BL)
        sel = np.zeros((B, BL), np.float32)
        sel[np.arange(r * BL, (r + 1) * BL), np.arange(BL)] = 1.0
        isel = np.concatenate([np.eye(B, dtype=np.float32), sel], axis=1)
        h2t0 = np.concatenate([h2t_full, h2t_full[:, :, own]], axis=2)
        maps.append(dict(
            common,
            h2t0=np.ascontiguousarray(h2t0).astype(bfnp),
            memo=memory[own].astype(bfnp),
            isel=np.ascontiguousarray(isel),
            fcw=np.ascontiguousarray(
                fc_w[:, r * VS:(r + 1) * VS]).reshape(KC, 128, VS).astype(bfnp),
            fcbr=np.ascontiguousarray(
                np.broadcast_to(fc_b[r * VS:(r + 1) * VS], (128, VS)), np.float32),
        ))
    return maps


def kernel(**inputs):
    if "nc" not in _CACHE:
        _CACHE["nc"] = _build()
    nc = _CACHE["nc"]
    maps = _prep_inputs(inputs)
    res = run_bass_kernel_spmd(nc, maps, list(range(NC)))
    global LAST_RESULT
    LAST_RESULT = res
    out = np.concatenate([res.results[r]["out"] for r in range(NC)], axis=2)
    return out


LAST_RESULT = None
